# revision 16
# baseline (speedup 1.0000x reference)
"""Trainium2 Bass kernel for nn_CopyGenerator (scatter_memory).

Strategy (8 NeuronCores, data-parallel over rows / batch — NO collectives):
  - Each core owns 256 rows (2 m-tiles of 128) x the FULL 32000 vocab, so the
    softmax denominator is local to the core: zero cross-core communication,
    zero sensitivity to launch skew between cores.
  - logits = hidden @ W.T + b as bf16 matmuls with an augmented contraction
    (K = 8x128 + 1 bias row); W is replicated and streamed from HBM exactly
    once per core (~66 MB, under the PE time at 2.4 GHz, so PE-bound).
  - Pass A (per 500-wide vocab tile): GEMM -> Exp -> bf16 slab resident in
    SBUF, softmax partial sums accumulated by the ACT engine (accum_out).
    Pass B: scatter-as-matmul (one-hot E), fused scale/add on DVE, Ln on ACT,
    fp16 store.  All Exp strictly before all Ln => exactly 2 ACT table loads.
  - DMA-queue discipline (the HWDGE ring costs ~650 ns fixed per DMA on top
    of transfer time): loads are merged into super-tile DMAs on the SP queue
    (32 W loads of 1 MB, 16 fused E+AT loads), stores are merged 4 vocab
    tiles wide and issued on the Activation-engine DGE queue.
  - The per-batch scatter-add of copy-attention mass is a dense matmul
    AE = A_T.T @ E with host-built operands (slot -> vocab one-hot); the
    copy gate c = sigmoid(l_copy) is folded into A_T on the host, and an
    extra always-on slot adds the reference's +EPS.
  - Per-row specials (1-c, exp(l_pad), c*pad_attn_mass) are exact f32
    host-side matvecs; on-device they only enter tiny [128,1] vector ops.

kernel(**inputs) takes FULL inputs, returns the FULL (2048, 32000) f32 output.
"""

import numpy as np
import ml_dtypes

EPS = 1e-10
N_CORES = 8
LB = 2048          # tgt_len * batch rows
D = 1024           # d_model
V = 32000          # vocab
B = 64             # batch
S = 64             # src len
RPC = LB // N_CORES  # rows per core (256)
MT = RPC // 128      # m-tiles per core (2)
NW = 500             # vocab tile width
NT = V // NW         # vocab tiles (64)
KS = 128             # scatter slot capacity per vocab tile (slot 127 = EPS)
KC = 8               # 128-row contraction chunks (plus 1 bias row)
SW = 2               # vocab tiles per W super-tile DMA
SA = 4               # vocab tiles per E+AT / out super-tile DMA
AEW = NW + RPC       # fused E+AT super-tile width per vocab tile (756)
BF16 = ml_dtypes.bfloat16

POOL_STT = True

_PROGRAM_CACHE = {}


def _build_program(pad_n, pad_c, single_core=False, compile_=True):
    """Build + compile the SPMD Bass program (identical on every core).
    pad_n/pad_c: vocab tile index and column of pad_idx. single_core: build a
    1-device variant for TimelineSim (program body is identical)."""
    import concourse.tile as tile
    from concourse import bacc, mybir

    f32 = mybir.dt.float32
    f16 = mybir.dt.float16
    bf16 = mybir.dt.bfloat16
    AX = mybir.AxisListType
    OP = mybir.AluOpType
    AF = mybir.ActivationFunctionType

    nc = bacc.Bacc("TRN2", target_bir_lowering=False, debug=False,
                   num_devices=1 if single_core else N_CORES)

    ht_ext = nc.dram_tensor("ht", [128, (KC + 1) * RPC], bf16,
                            kind="ExternalInput")
    wtt_ext = nc.dram_tensor("wtt", [NT // SW, 128, SW * (KC + 1) * NW], bf16,
                             kind="ExternalInput")
    ae_ext = nc.dram_tensor("ae", [NT // SA, KS, SA * AEW], bf16,
                            kind="ExternalInput")
    sca_ext = nc.dram_tensor("sca", [128, MT * 4], f32, kind="ExternalInput")
    out_ext = nc.dram_tensor("out", [RPC, V], f16, kind="ExternalOutput")

    with tile.TileContext(nc) as tc:
        with (
            tc.tile_pool(name="const", bufs=1) as const,
            tc.tile_pool(name="slabs", bufs=1) as slabs,
            tc.tile_pool(name="wpool", bufs=2) as wpool,
            tc.tile_pool(name="aep", bufs=2) as aep,
            tc.tile_pool(name="statp", bufs=8) as statp,
            tc.tile_pool(name="bigp", bufs=4) as bigp,
            tc.tile_pool(name="obp", bufs=2) as obp,
            tc.tile_pool(name="psl", bufs=3, space="PSUM") as psl,
            tc.tile_pool(name="psa", bufs=3, space="PSUM") as psa,
        ):
            # ---- residents ----
            ht_sb = const.tile([128, (KC + 1) * RPC], bf16, name="ht_sb")
            nc.sync.dma_start(ht_sb[:], ht_ext.ap())
            sca_sb = const.tile([128, MT * 4], f32, name="sca_sb")
            nc.sync.dma_start(sca_sb[:], sca_ext.ap())

            slab = [slabs.tile([128, V], bf16, name=f"slab{m}")
                    for m in range(MT)]
            pstat = [statp.tile([128, NT], f32, tag="pstat", name=f"pstat{m}")
                     for m in range(MT)]

            # ---- pass A: logits matmul + Exp -> slab, Z partials ----
            for st in range(NT // SW):
                w = wpool.tile([128, SW * (KC + 1) * NW], bf16, tag="w",
                               name=f"w{st}")
                nc.sync.dma_start(w[:], wtt_ext[st])
                for nl in range(SW):
                    n = st * SW + nl
                    base = nl * (KC + 1) * NW
                    for m in range(MT):
                        pl = psl.tile([128, NW], f32, tag="psl",
                                      name=f"psl{n}_{m}")
                        for kc in range(KC):
                            nc.tensor.matmul(
                                pl[:],
                                ht_sb[:, kc * RPC + m * 128:
                                      kc * RPC + (m + 1) * 128],
                                w[:, base + kc * NW:base + (kc + 1) * NW],
                                start=(kc == 0), stop=False)
                        nc.tensor.matmul(
                            pl[:],
                            ht_sb[0:1, KC * RPC + m * 128:
                                  KC * RPC + (m + 1) * 128],
                            w[0:1, base + KC * NW:base + (KC + 1) * NW],
                            start=False, stop=True)
                        nc.scalar.activation(slab[m][:, n * NW:(n + 1) * NW],
                                             pl[:], AF.Exp,
                                             accum_out=pstat[m][:, n:n + 1])

            # ---- per-row scalars ----
            s1 = {}
            invs = {}
            fix = {}
            for m in range(MT):
                omc = sca_sb[:, m * 4 + 0:m * 4 + 1]
                elp = sca_sb[:, m * 4 + 1:m * 4 + 2]
                csc0 = sca_sb[:, m * 4 + 2:m * 4 + 3]
                zacc = statp.tile([128, 1], f32, tag="zacc", name=f"zacc{m}")
                nc.vector.tensor_reduce(zacc[:], pstat[m][:], axis=AX.X,
                                        op=OP.add)
                z = statp.tile([128, 1], f32, tag="z", name=f"z{m}")
                nc.vector.tensor_add(z[:], zacc[:], elp)
                nc.vector.tensor_scalar_add(z[:], z[:], -1.0)
                invz = statp.tile([128, 1], f32, tag="invz", name=f"invz{m}")
                nc.vector.reciprocal(invz[:], z[:])
                t_s1 = statp.tile([128, 1], f32, tag="s1", name=f"s1_{m}")
                nc.vector.tensor_mul(t_s1[:], invz[:], omc)
                u1 = statp.tile([128, 1], f32, tag="u1", name=f"u1_{m}")
                nc.vector.tensor_mul(u1[:], t_s1[:], elp)
                u2 = statp.tile([128, 1], f32, tag="u2", name=f"u2_{m}")
                nc.vector.tensor_add(u2[:], u1[:], csc0)
                sg = statp.tile([128, 1], f32, tag="sg", name=f"sg{m}")
                nc.vector.tensor_scalar(sg[:], u2[:], -1.0, 1.0 + EPS,
                                        op0=OP.mult, op1=OP.add)
                t_invs = statp.tile([128, 1], f32, tag="invs", name=f"invs{m}")
                nc.vector.reciprocal(t_invs[:], sg[:])
                t_fix = statp.tile([128, 1], f32, tag="fix", name=f"fix{m}")
                nc.vector.tensor_scalar(t_fix[:], sg[:], EPS, EPS,
                                        op0=OP.mult, op1=OP.add)
                s1[m] = t_s1
                invs[m] = t_invs
                fix[m] = t_fix

            # ---- pass B: scatter matmul, fuse, Ln, merged fp16 store ----
            for st in range(NT // SA):
                ae = aep.tile([KS, SA * AEW], bf16, tag="ae", name=f"ae{st}")
                nc.sync.dma_start(ae[:], ae_ext[st])
                osb = [obp.tile([128, SA * NW], f16, tag=f"osb{m}",
                                name=f"osb{st}_{m}") for m in range(MT)]
                for nl in range(SA):
                    n = st * SA + nl
                    e_sl = ae[:, nl * AEW:nl * AEW + NW]
                    for m in range(MT):
                        at_sl = ae[:, nl * AEW + NW + m * 128:
                                 nl * AEW + NW + (m + 1) * 128]
                        pa = psa.tile([128, NW], f32, tag="psa",
                                      name=f"pa{n}_{m}")
                        nc.tensor.matmul(pa[:], at_sl, e_sl,
                                         start=True, stop=True)
                        sb3 = bigp.tile([128, NW], f32, tag="sb3",
                                        name=f"sb3_{n}_{m}")
                        eng = (nc.gpsimd if POOL_STT and (n % 3 == 2)
                               else nc.vector)
                        eng.scalar_tensor_tensor(
                            sb3[:], slab[m][:, n * NW:(n + 1) * NW],
                            s1[m][:], pa[:], op0=OP.mult, op1=OP.add)
                        if n == pad_n:
                            nc.vector.tensor_copy(sb3[:, pad_c:pad_c + 1],
                                                  fix[m][:])
                        nc.scalar.activation(
                            osb[m][:, nl * NW:(nl + 1) * NW], sb3[:], AF.Ln,
                            scale=invs[m][:])
                for m in range(MT):
                    nc.gpsimd.dma_start(
                        out_ext[m * 128:(m + 1) * 128,
                                st * SA * NW:(st + 1) * SA * NW],
                        osb[m][:])

    if compile_:
        nc.compile()
    return nc


def _host_prep(hidden, attn, W, b, src, alignment, copy_idx, pad_idx):
    hidden = np.asarray(hidden, np.float32)
    attn = np.asarray(attn, np.float32)
    W = np.asarray(W, np.float32)
    b = np.asarray(b, np.float32)
    src = np.asarray(src)
    alignment = np.asarray(alignment)
    copy_idx = int(copy_idx)
    pad_idx = int(pad_idx)

    tgt = alignment[src[:, :, 0]].T.astype(np.int64)   # (B, S)

    # per-row specials, exact in f32 on host
    l_copy = hidden @ W[copy_idx] + b[copy_idx]        # (LB,)
    l_pad = hidden @ W[pad_idx] + b[pad_idx]           # (LB,)
    c = 1.0 / (1.0 + np.exp(-l_copy))
    omc = 1.0 - c
    elp = np.exp(l_pad)

    sc0 = np.zeros(LB, np.float32)
    pad_mask = tgt == pad_idx                          # (B, S)
    for bb in range(B):
        if pad_mask[bb].any():
            sc0[bb::B] = attn[bb::B][:, pad_mask[bb]].sum(axis=1)
    csc0 = c * sc0

    # augmented weights: W.T chunks + bias row; special cols zeroed
    W_t = W.T.copy()                                   # (D, V)
    bias = b.copy()
    W_t[:, copy_idx] = 0.0
    bias[copy_idx] = EPS
    W_t[:, pad_idx] = 0.0
    bias[pad_idx] = 0.0
    # wtt[st, p, nl*(KC+1)*NW + kc*NW + c] = W_t[kc*128+p, (st*SW+nl)*NW+c];
    # the kc==KC block carries the bias row on partition 0
    wtt = np.zeros((NT // SW, 128, SW * (KC + 1) * NW), np.float32)
    wv = W_t.reshape(KC, 128, NT // SW, SW, NW).transpose(2, 1, 3, 0, 4)
    bv = bias.reshape(NT // SW, SW, NW)
    for nl in range(SW):
        base = nl * (KC + 1) * NW
        wtt[:, :, base:base + KC * NW] = \
            wv[:, :, nl].reshape(NT // SW, 128, KC * NW)
        wtt[:, 0, base + KC * NW:base + (KC + 1) * NW] = bv[:, nl]
    wtt = wtt.astype(BF16)

    hT = hidden.T                                      # (D, LB)

    # scatter operands: E one-hot + AT attn columns (c folded), fused per
    # SA-super-tile: ae[st, j, nl*AEW + (0:NW)] = E, (NW:NW+RPC) = AT rows
    AT = np.zeros((NT, KS, LB), np.float32)
    E = np.zeros((NT, KS, NW), np.float32)
    counts = np.zeros(NT, np.int64)
    bs, ss = np.nonzero(tgt != pad_idx)
    for bb, s in zip(bs, ss):
        tv = tgt[bb, s]
        t = tv // NW
        j = counts[t]
        assert j < KS - 1, f"scatter slot overflow: tile {t}"
        counts[t] = j + 1
        AT[t, j, bb::B] = attn[bb::B, s] * c[bb::B]
        E[t, j, tv % NW] = 1.0
    # EPS slot: adds +EPS to every output column (reference's log(x + EPS))
    AT[:, KS - 1, :] = EPS
    E[:, KS - 1, :] = 1.0

    in_maps = []
    for k in range(N_CORES):
        lo = k * RPC
        ht_core = np.empty((128, (KC + 1) * RPC), np.float32)
        for kc in range(KC):
            ht_core[:, kc * RPC:(kc + 1) * RPC] = \
                hT[kc * 128:(kc + 1) * 128, lo:lo + RPC]
        ht_core[:, KC * RPC:] = 0.0
        ht_core[0, KC * RPC:] = 1.0                    # bias ones row
        ae = np.empty((NT // SA, KS, SA * AEW), np.float32)
        for st in range(NT // SA):
            for nl in range(SA):
                n = st * SA + nl
                ae[st, :, nl * AEW:nl * AEW + NW] = E[n]
                ae[st, :, nl * AEW + NW:(nl + 1) * AEW] = \
                    AT[n, :, lo:lo + RPC]
        sca = np.zeros((128, MT * 4), np.float32)
        for m in range(MT):
            rows = slice(lo + m * 128, lo + (m + 1) * 128)
            sca[:, m * 4 + 0] = omc[rows]
            sca[:, m * 4 + 1] = elp[rows]
            sca[:, m * 4 + 2] = csc0[rows]
        in_maps.append({
            "ht": ht_core.astype(BF16),
            "wtt": wtt,
            "ae": ae.astype(BF16),
            "sca": sca,
        })
    pad_n = pad_idx // NW
    pad_c = pad_idx % NW
    return in_maps, pad_n, pad_c


def _run(in_maps, pad_n, pad_c, trace=False):
    from concourse.bass_utils import run_bass_kernel_spmd
    key = (pad_n, pad_c)
    if key not in _PROGRAM_CACHE:
        _PROGRAM_CACHE[key] = _build_program(pad_n, pad_c)
    nc = _PROGRAM_CACHE[key]
    res = run_bass_kernel_spmd(nc, in_maps, list(range(N_CORES)), trace=trace)
    return res


def kernel(hidden, attn, W, b, src, alignment, copy_idx=4, pad_idx=0,
           _trace=False, _return_raw=False):
    in_maps, pad_n, pad_c = _host_prep(hidden, attn, W, b, src, alignment,
                                       copy_idx, pad_idx)
    res = _run(in_maps, pad_n, pad_c, trace=_trace)
    out = np.concatenate(
        [res.results[k]["out"].astype(np.float32) for k in range(N_CORES)],
        axis=0)
    if _return_raw:
        return out, res
    return out


# ---------------------------------------------------------------------------
# Benchmarking support (test.py only): async-pipelined dispatch, difference
# vs a null kernel with identical output shape.  Resolution is limited by the
# per-call RPC floor (~2 ms); TimelineSim (sim.py) is the precise dev metric.
# ---------------------------------------------------------------------------

def _make_async_runner(nc, in_maps):
    import jax
    from jax.sharding import Mesh, PartitionSpec, NamedSharding
    from jax.experimental.shard_map import shard_map
    from concourse import bass2jax, mybir

    bass2jax.install_neuronx_cc_hook()
    partition_name = (nc.partition_id_tensor.name
                      if nc.partition_id_tensor else None)
    in_names, out_names, out_avals, zero_outs = [], [], [], []
    for alloc in nc.m.functions[0].allocations:
        if not isinstance(alloc, mybir.MemoryLocationSet):
            continue
        name = alloc.memorylocations[0].name
        if alloc.kind == "ExternalInput":
            if name != partition_name:
                in_names.append(name)
        elif alloc.kind == "ExternalOutput":
            out_names.append(name)
            shape = tuple(alloc.tensor_shape)
            dtype = mybir.dt.np(alloc.dtype)
            out_avals.append(jax.core.ShapedArray(shape, dtype))
            zero_outs.append(np.zeros(shape, dtype))
    n_params = len(in_names)
    in_names = in_names + out_names
    if partition_name is not None:
        in_names.append(partition_name)

    def _body(*args):
        ins = list(args[:n_params])
        outs = tuple(args[n_params:])
        pid = ([bass2jax.partition_id_tensor()]
               if partition_name is not None else [])
        return tuple(bass2jax._bass_exec_p.bind(
            *ins, *outs, *pid, out_avals=tuple(out_avals),
            in_names=tuple(in_names), out_names=tuple(out_names),
            lowering_input_output_aliases=(), sim_require_finite=True,
            sim_require_nnan=True, nc=nc))

    n = len(in_maps)
    devices = jax.devices()[:n]
    mesh = Mesh(np.asarray(devices), ("core",))
    spec = PartitionSpec("core")
    sharding = NamedSharding(mesh, spec)
    in_specs = (spec,) * (n_params + len(out_names))
    out_specs = (spec,) * len(out_names)
    fn = jax.jit(shard_map(_body, mesh=mesh, in_specs=in_specs,
                           out_specs=out_specs, check_rep=False),
                 keep_unused=True)
    per_core = [[np.asarray(m[name]) for name in in_names[:n_params]]
                for m in in_maps]
    args = [jax.device_put(
        np.concatenate([per_core[c][i] for c in range(n)], axis=0), sharding)
        for i in range(n_params)]
    args += [jax.device_put(
        np.zeros((n * z.shape[0], *z.shape[1:]), z.dtype), sharding)
        for z in zero_outs]
    return fn, args


def _build_null_program():
    """Trivial SPMD NEFF with the same output shape (launch/alloc control)."""
    import concourse.tile as tile
    from concourse import bacc, mybir
    f32 = mybir.dt.float32
    f16 = mybir.dt.float16
    nc = bacc.Bacc("TRN2", target_bir_lowering=False, debug=False,
                   num_devices=N_CORES)
    x = nc.dram_tensor("x", [128, 128], f32, kind="ExternalInput")
    y = nc.dram_tensor("out", [RPC, V], f16, kind="ExternalOutput")
    with tile.TileContext(nc) as tc:
        with tc.tile_pool(name="p", bufs=1) as p:
            t = p.tile([128, 128], f32)
            nc.sync.dma_start(t[:], x.ap())
            o = p.tile([128, 128], f16)
            nc.vector.tensor_copy(o[:], t[:])
            nc.sync.dma_start(y[0:128, 0:128], o[:])
    nc.compile()
    return nc


def benchmark(hidden, attn, W, b, src, alignment, copy_idx=4, pad_idx=0,
              iters=4, M=48):
    """Async-pipelined per-call estimate: (kernel/call - null/call) at M
    in-flight dispatches.  Returns (est_hw_ns, t_kernel_list, t_null_list)."""
    import time
    import jax
    in_maps, pad_n, pad_c = _host_prep(hidden, attn, W, b, src, alignment,
                                       copy_idx, pad_idx)
    key = (pad_n, pad_c)
    if key not in _PROGRAM_CACHE:
        _PROGRAM_CACHE[key] = _build_program(pad_n, pad_c)
    nc = _PROGRAM_CACHE[key]
    fn_k, args_k = _make_async_runner(nc, in_maps)
    null_nc = _build_null_program()
    null_maps = [{"x": np.zeros((128, 128), np.float32)}
                 for _ in range(N_CORES)]
    fn_n, args_n = _make_async_runner(null_nc, null_maps)

    def timed(fn, args):
        outs = fn(*args)
        jax.block_until_ready(outs)
        ts = []
        for _ in range(iters):
            t0 = time.perf_counter()
            res = [fn(*args) for _ in range(M)]
            jax.block_until_ready(res)
            ts.append((time.perf_counter() - t0) / M)
        return ts

    t_k = timed(fn_k, args_k)
    t_n = timed(fn_n, args_n)
    est = max(0.0, min(t_k) - min(t_n))
    return int(est * 1e9), t_k, t_n
